# revision 71
# baseline (speedup 1.0000x reference)
"""Multi-head attention (16 heads, d_model=1024, T=2048, B=2) on 8 trn2 NeuronCores.

Sharding: core c -> batch c//4, head-group c%4 (4 heads of 64 dims each).
Each core computes q/k/v projections for its 4 heads on its batch, full
softmax attention for those heads, and a partial output projection
(row-parallel Wo).  Host sums the 4 partials per batch and adds the bias.

v2 design (vs baseline):
  - score matmuls (contract=64) emitted as adjacent h0/h64 row-tile pairs
    -> run concurrently on the two 64x128 PE tiles (~2x).
  - attn*V matmuls col-tiled: h0 -> psum partitions 0-63, h1 -> 64-127 of
    the same bank (~2x, and AV psum shrinks to 2 banks).
  - softmax denominators: fp16 running accumulation of the exp tiles on
    DVE/GpSimd, then a single ones-vector matmul per (head, chunk) reduces
    the 128 partitions; 1/D via ACT Ln + Exp(-x) (same act table set as
    the softmax Exp, so no table reloads).
  - V projection computed directly in [u, s] orientation (stationary xT
    block, moving Wv) - no PE transposes.
  - static software pipeline: projection / output chunks are interleaved
    into the ACT(exp)-bound attention phases as fillers; each phase's
    normalization is emitted as the first filler of the next phase so the
    PE never waits on the 1/D chain.
PSUM budget: st 2x[128,1024] (4 banks) + av 2x[128,512] (2) + pj 2 (2).
"""

import math
import os as _os

import numpy as np
import ml_dtypes

B = 2
T = 2048
K = 1024
H = 16
S = K // H  # 64
NCORES = 8
TB = T // 128  # 16 u-blocks
KT = K // 128  # 8 contraction tiles

_BF16 = ml_dtypes.bfloat16
_GPS_ACC = _os.environ.get("GPS_ACC", "0") == "1"


def _install_drain_split_patch():
    """walrus in this container rejects >1 sync-wait on the final tile drain;
    split the waits one-per-drain-instruction (all before the end barrier)."""
    import concourse.tile as tile
    import concourse.mybir as mybir
    from concourse.vector_clock import ScopedClock

    if getattr(tile.TileContext, "_drain_split_patched", False):
        return

    def _patched_dab(self, tick_clock, wait_clock):
        drain_inst = self.nc.sync.drain()
        wait_clock.add_sem_waits(
            drain_inst.ins, ScopedClock({None: tick_clock.global_clock})
        )
        si = drain_inst.ins.sync_info
        waits = list(si.on_wait) if si is not None else []
        if len(waits) > 1:
            si.on_wait = waits[:1]
            for w in waits[1:]:
                extra = self.nc.sync.drain()
                esi = extra.ins.sync_info
                if esi is None:
                    extra.ins.sync_info = mybir.SyncInfo(on_update=[], on_wait=[w])
                else:
                    esi.on_wait = [w]
        self.nc.all_engine_barrier()
        assert self.sems is not None
        popped = self.nc._tile_sem_poison_stack.pop()
        assert popped is self._sem_poison
        self.nc.clear_and_free_semaphores(list(self.sems.allocated().values()))
        self.nc.all_engine_barrier()

    tile.TileContext._drain_and_barrier = _patched_dab
    tile.TileContext._drain_split_patched = True


def build_program():
    import concourse.bass as bass
    import concourse.mybir as mybir
    import concourse.tile as tile
    from concourse import bacc

    dt = mybir.dt
    AF = mybir.ActivationFunctionType
    Alu = mybir.AluOpType

    nc = bacc.Bacc()

    xT = nc.dram_tensor("xT", [K, T], dt.bfloat16, kind="ExternalInput")
    wq = nc.dram_tensor("wq", [K, 256], dt.bfloat16, kind="ExternalInput")
    wk = nc.dram_tensor("wk", [K, 256], dt.bfloat16, kind="ExternalInput")
    wv = nc.dram_tensor("wv", [K, 256], dt.bfloat16, kind="ExternalInput")
    wo = nc.dram_tensor("wo", [256, K], dt.bfloat16, kind="ExternalInput")
    out = nc.dram_tensor("out", [T, K], dt.float16, kind="ExternalOutput")

    with tile.TileContext(nc) as tc:
        with (
            tc.tile_pool(name="xt", bufs=KT) as xt_pool,
            tc.tile_pool(name="w", bufs=3) as w_pool,
            tc.tile_pool(name="wo", bufs=2) as wo_pool,
            tc.tile_pool(name="qk", bufs=4) as qk_pool,
            tc.tile_pool(name="v", bufs=1) as v_pool,
            tc.tile_pool(name="es", bufs=6) as es_pool,
            tc.tile_pool(name="acc", bufs=8) as acc_pool,
            tc.tile_pool(name="dd", bufs=2) as dd_pool,
            tc.tile_pool(name="yt", bufs=2) as yt_pool,
            tc.tile_pool(name="osb", bufs=2) as osb_pool,
            tc.tile_pool(name="st", bufs=2, space="PSUM") as st_pool,
            tc.tile_pool(name="av", bufs=2, space="PSUM") as av_pool,
            tc.tile_pool(name="pj", bufs=2, space="PSUM") as pj_pool,
        ):
            # ---------------- loads ----------------
            w_sb = {}
            for name, dram in (("q", wq), ("k", wk), ("v", wv)):
                t = w_pool.tile([128, KT * 256], dt.bfloat16, tag="w", name=f"w_{name}")
                nc.sync.dma_start(
                    t[:].rearrange("p (a c) -> p a c", a=KT),
                    dram.rearrange("(a p) c -> p a c", p=128),
                )
                w_sb[name] = t

            xt = []
            for a in range(KT):
                t = xt_pool.tile([128, T], dt.bfloat16, tag="xt", name=f"xt_{a}")
                nc.sync.dma_start(t[:], xT[a * 128 : (a + 1) * 128, :])
                xt.append(t)

            wo_sb = []
            for i in range(2):
                t = wo_pool.tile([128, K], dt.bfloat16, tag="wo", name=f"wo_{i}")
                nc.sync.dma_start(t[:], wo[i * 128 : (i + 1) * 128, :])
                wo_sb.append(t)

            # selector matrices: broadcast 1/D rows (at partitions 0/32/64/96)
            # over the 64-partition bands of the head pair, per 512-chunk c.
            sel = []
            for c in range(2):
                s = v_pool.tile([128, 128], dt.bfloat16, tag=f"sel{c}", name=f"sel_{c}")
                nc.vector.memset(s[:], 0.0)
                nc.vector.memset(s[32 * c : 32 * c + 1, 0:64], 1.0)
                nc.vector.memset(s[64 + 32 * c : 64 + 32 * c + 1, 64:128], 1.0)
                sel.append(s)

            # denominator selectors: ones column at 0 (h0 -> out row 0) or
            # 64 (h1 -> out row 64), zero elsewhere -> (128,128)-mode reduce
            dsel = []
            for hl in range(2):
                s = v_pool.tile(
                    [128, 128], dt.bfloat16, tag=f"dsel{hl}", name=f"dsel_{hl}"
                )
                nc.vector.memset(s[:], 0.0)
                nc.vector.memset(s[:, hl * 64 : hl * 64 + 1], 1.0)
                dsel.append(s)

            # PE warm-up: dummy matmuls on the (memset-only) selector tiles
            # while input DMAs land, so the HAM clock gate opens before the
            # first real matmul instead of ~20us into the kernel.
            warm_src = v_pool.tile([128, 512], dt.bfloat16, tag="warm", name="warm_src")
            nc.vector.memset(warm_src[:], 0.0)

            def warm_mms(n, label):
                # dummy matmuls keep the HAM clock gate open while the PE
                # would otherwise idle (input-DMA window, finalize chains)
                for i in range(n):
                    wps = pj_pool.tile(
                        [128, 512], dt.float32, tag="pj", name=f"warm_{label}_{i}"
                    )
                    nc.tensor.matmul(
                        wps[:], sel[0][:], warm_src[:], start=True, stop=True
                    )

            warm_mms(20, "pre")



            # V stationaries, zero-padded to M=128 per (ub, hp):
            # block layout [v_h0 (64) | zeros (64) | v_h1 (64)] of 192 cols so
            # h0 reads cols 0:128 = [v|0], h1 reads cols 64:192 = [0|v].
            VBLK = 192
            v_sb = v_pool.tile([128, TB * 2 * VBLK], dt.bfloat16, tag="v", name="v_sb")
            nc.gpsimd.memset(v_sb[:], 0.0)

            # V^T staging ([s, u] orientation straight out of the projection;
            # DMA-transpose moves blocks into v_sb)
            vt_sb = [
                v_pool.tile([128, T], dt.bfloat16, tag=f"vt{h}", name=f"vt_{h}")
                for h in range(2)
            ]

            qt_sb = [None, None]
            ktp_sb = {}  # (hp, hl) -> partition-padded K^T tile
            yt_sb = [
                yt_pool.tile([128, T], dt.bfloat16, tag="yt", name=f"yt_{hp}")
                for hp in range(2)
            ]

            # ---------------- projection chunks (fillers) ----------------
            def v_chunk(hp, cp):
                """V^T projection for head pair hp, column pair cp (u-columns
                cp*1024..+1024 = u-blocks 8cp..8cp+8), weight-stationary like
                qk_chunk; then DMA-transpose blocks into the padded v_sb.
                Generator: yields every 2 contraction steps (~1us of PE)."""
                ps = [
                    pj_pool.tile(
                        [128, 512], dt.float32, tag="pj", name=f"pv_{hp}_{cp}_{i}"
                    )
                    for i in range(2)
                ]
                for a in range(KT):
                    for i in range(2):
                        nc.tensor.matmul(
                            ps[i][:],
                            w_sb["v"][:, a * 256 + hp * 128 : a * 256 + hp * 128 + 128],
                            xt[a][:, cp * 1024 + i * 512 : cp * 1024 + (i + 1) * 512],
                            start=(a == 0),
                            stop=(a == KT - 1),
                        )
                    if a % 2 == 1 and a < KT - 1:
                        yield
                for i in range(2):
                    tsl = slice(cp * 1024 + i * 512, cp * 1024 + (i + 1) * 512)
                    nc.vector.tensor_copy(vt_sb[hp][:, tsl], ps[i][:])
                for ub in range(cp * 8, cp * 8 + 8):
                    for hl in range(2):
                        nc.sync.dma_start(
                            v_sb[
                                :,
                                ub * 2 * VBLK + hp * VBLK + hl * 128 : ub * 2 * VBLK + hp * VBLK + hl * 128 + 64,
                            ],
                            vt_sb[hp][hl * 64 : (hl + 1) * 64, ub * 128 : (ub + 1) * 128],
                            transpose=True,
                        )

            def qk_chunk(hp, which, cp):
                """q or k projection for head pair hp, column pair cp
                (2 x 512 t-columns); stationary weight slice shared across
                the c pair, accumulating a=0..7 into two pj banks.
                K evacuates into two partition-padded tiles (other head's
                64 partitions zeroed) so score matmuls run at full 128
                contract in uniform (128,128) mode."""
                if which == "q" and qt_sb[hp] is None:
                    qt_sb[hp] = qk_pool.tile(
                        [128, T], dt.bfloat16, tag="qk", name=f"qt_{hp}"
                    )
                if which == "k" and (hp, 0) not in ktp_sb:
                    for hl in range(2):
                        t = qk_pool.tile(
                            [128, T], dt.bfloat16, tag=f"ktp{hl}", name=f"ktp_{hp}_{hl}"
                        )
                        nc.gpsimd.memset(t[(1 - hl) * 64 : (2 - hl) * 64, :], 0.0)
                        ktp_sb[(hp, hl)] = t
                ps = [
                    pj_pool.tile(
                        [128, 512], dt.float32, tag="pj", name=f"p{which}_{hp}_{cp}_{i}"
                    )
                    for i in range(2)
                ]
                for a in range(KT):
                    for i in range(2):
                        nc.tensor.matmul(
                            ps[i][:],
                            w_sb[which][:, a * 256 + hp * 128 : a * 256 + hp * 128 + 128],
                            xt[a][:, cp * 1024 + i * 512 : cp * 1024 + (i + 1) * 512],
                            start=(a == 0),
                            stop=(a == KT - 1),
                        )
                    if a % 2 == 1 and a < KT - 1:
                        yield
                for i in range(2):
                    tsl = slice(cp * 1024 + i * 512, cp * 1024 + (i + 1) * 512)
                    if which == "q":
                        nc.vector.tensor_copy(qt_sb[hp][:, tsl], ps[i][:])
                    else:
                        for hl in range(2):
                            psl = slice(hl * 64, (hl + 1) * 64)
                            nc.vector.tensor_copy(
                                ktp_sb[(hp, hl)][psl, tsl], ps[i][psl, :]
                            )

            def out_chunk(tb, use_act=False):
                """output projection for t-block tb; evac; DMA out.  In the
                tail (use_act) ACT is idle and takes one of the two evacs."""
                osb = osb_pool.tile([128, K], dt.float16, tag="osb", name=f"osb_{tb}")
                pso = [
                    pj_pool.tile([128, 512], dt.float32, tag="pj", name=f"po_{tb}_{i}")
                    for i in range(2)
                ]
                for hp in range(2):
                    for oc in range(2):
                        nc.tensor.matmul(
                            pso[oc][:],
                            yt_sb[hp][:, tb * 128 : (tb + 1) * 128],
                            wo_sb[hp][:, oc * 512 : (oc + 1) * 512],
                            start=(hp == 0),
                            stop=(hp == 1),
                        )
                nc.vector.tensor_copy(osb[:, 0:512], pso[0][:])
                if use_act:
                    nc.scalar.copy(osb[:, 512:1024], pso[1][:])
                else:
                    nc.vector.tensor_copy(osb[:, 512:1024], pso[1][:])
                nc.sync.dma_start(out[tb * 128 : (tb + 1) * 128, :], osb[:])

            # ---------------- attention phase ----------------
            def attention_phase(hp, th, fillers):
                """One (head pair, t-half) block.  fillers: callables emitting
                one chunk each, pumped one per ub iteration.  Returns a
                finalize closure (normalization) to pump into the NEXT phase
                (it must be emitted before that phase's first AV matmul)."""
                t0 = th * 1024
                qt = qt_sb[hp]
                av = [
                    av_pool.tile(
                        [128, 512], dt.float32, tag="av", name=f"av_{hp}_{th}_{c}"
                    )
                    for c in range(2)
                ]
                acc = {}

                def scores(ub):
                    sts = [
                        st_pool.tile(
                            [128, 1024], dt.float32, tag="st",
                            name=f"st_{hp}_{th}_{ub}_{hl}",
                        )
                        for hl in range(2)
                    ]
                    # full-contract (zero-padded) -> uniform (128,128) mode
                    for c in range(2):
                        for hl in range(2):
                            nc.tensor.matmul(
                                sts[hl][:, c * 512 : (c + 1) * 512],
                                ktp_sb[(hp, hl)][:, ub * 128 : (ub + 1) * 128],
                                qt[:, t0 + c * 512 : t0 + (c + 1) * 512],
                                start=True,
                                stop=True,
                            )
                    return sts

                def expify(ub, sts):
                    ess = []
                    for hl in range(2):
                        e = es_pool.tile(
                            [128, 1024], dt.bfloat16, tag="es",
                            name=f"e_{hp}_{th}_{ub}_{hl}",
                        )
                        nc.scalar.activation(e[:], sts[hl][:], AF.Exp)
                        ess.append(e)
                    return ess

                def avmm(ub, ess):
                    # col-tiled (128,64) pairs: h0 -> psum partitions 0-63,
                    # h1 -> 64-127 of the same bank.  Both es operands are a
                    # full iteration old, so the (c,h0)/(c,h1) pairs become
                    # ready together, stay adjacent, and run concurrently.
                    for c in range(2):
                        for hl in range(2):
                            vcol = ub * 2 * VBLK + hp * VBLK + hl * 128
                            nc.tensor.matmul(
                                av[c][hl * 64 : (hl + 1) * 64, :],
                                v_sb[:, vcol : vcol + 64],
                                ess[hl][:, c * 512 : (c + 1) * 512],
                                start=(ub == 0),
                                stop=(ub == TB - 1),
                                skip_group_check=True,
                            )

                def accum(ub, ess):
                    # two bf16 chains (even/odd ubs) keep rounding error ~0.2%
                    for hl in range(2):
                        ch = ub % 2
                        esl = ess[hl][:]
                        eng = nc.gpsimd if (_GPS_ACC and hl == 1) else nc.vector
                        if ub < 2:
                            a0 = acc_pool.tile(
                                [128, 1024], dt.bfloat16, tag="acc",
                                name=f"acc_{hp}_{th}_{hl}_{ch}_0",
                            )
                            eng.tensor_copy(a0[:], esl)
                            acc[(hl, ch)] = a0
                        else:
                            nxt = acc_pool.tile(
                                [128, 1024], dt.bfloat16, tag="acc",
                                name=f"acc_{hp}_{th}_{hl}_{ch}_{ub}",
                            )
                            eng.tensor_tensor(
                                nxt[:], acc[(hl, ch)][:], esl, op=Alu.add
                            )
                            acc[(hl, ch)] = nxt

                # filler pump: ~2 quanta (~1us of PE) per ub slot, so chunky
                # projection work never dams the score->exp chain
                pend = list(fillers)
                active = [None]

                def pump_slot(budget=2):
                    while budget > 0:
                        if active[0] is not None:
                            try:
                                next(active[0])
                                budget -= 1
                                continue
                            except StopIteration:
                                active[0] = None
                        if not pend:
                            return
                        r = pend.pop(0)()
                        if hasattr(r, "__next__"):
                            active[0] = r
                        else:
                            budget -= 2

                # software pipeline: AV lags scores by one ub
                prev = None
                for ub in range(TB):
                    sts = scores(ub)
                    ess = expify(ub, sts)
                    if prev is not None:
                        avmm(prev[0], prev[1])
                        accum(prev[0], prev[1])
                    pump_slot()
                    prev = (ub, ess)
                avmm(prev[0], prev[1])
                accum(prev[0], prev[1])
                while pend or active[0] is not None:
                    pump_slot(budget=100)

                # merge the even/odd accumulation chains on DVE (PE is the
                # bottleneck; this halves the selector-matmul count)
                accm = {}
                for hl in range(2):
                    m = acc_pool.tile(
                        [128, 1024], dt.bfloat16, tag="acc", name=f"accm_{hp}_{th}_{hl}"
                    )
                    nc.vector.tensor_tensor(
                        m[:], acc[(hl, 0)][:], acc[(hl, 1)][:], op=Alu.add
                    )
                    accm[hl] = m

                # denominators: padded-selector matmul partition reduction
                # (D_h0 -> out row 0, D_h1 -> row 64, same bank, same mode)
                pjd = [
                    pj_pool.tile(
                        [128, 512], dt.float32, tag="pj", name=f"pjd_{hp}_{th}_{c}"
                    )
                    for c in range(2)
                ]
                for c in range(2):
                    for hl in range(2):
                        nc.tensor.matmul(
                            pjd[c][:],
                            dsel[hl][:],
                            accm[hl][:, c * 512 : (c + 1) * 512],
                            start=(hl == 0),
                            stop=(hl == 1),
                            skip_group_check=True,
                        )
                drows = dd_pool.tile(
                    [128, 512], dt.float32, tag="drows", name=f"drows_{hp}_{th}"
                )
                nc.gpsimd.memset(drows[:], 1.0)
                for c in range(2):
                    for hl in range(2):
                        p = hl * 64 + c * 32
                        nc.vector.tensor_copy(
                            drows[p : p + 1, :], pjd[c][hl * 64 : hl * 64 + 1, :]
                        )
                # 1/D via single-op approx reciprocal (~18 bits, plenty)
                dinv = dd_pool.tile(
                    [128, 512], dt.float32, tag="dln", name=f"dinv_{hp}_{th}"
                )
                nc.vector.reciprocal_approx_fast(dinv[:], drows[:])
                dinvb = dd_pool.tile(
                    [128, 512], dt.bfloat16, tag="dinvb", name=f"dinvb_{hp}_{th}"
                )
                nc.vector.tensor_copy(dinvb[:], dinv[:])

                def finalize():
                    # broadcast 1/D over partition bands via sel matmul,
                    # then yt = av * (1/D)
                    for c in range(2):
                        dbp = pj_pool.tile(
                            [128, 512], dt.float32, tag="pj", name=f"dbp_{hp}_{th}_{c}"
                        )
                        nc.tensor.matmul(
                            dbp[:], sel[c][:], dinvb[:], start=True, stop=True
                        )
                        dbs = dd_pool.tile(
                            [128, 512], dt.float32, tag="dbs", name=f"dbs_{hp}_{th}_{c}"
                        )
                        nc.vector.tensor_copy(dbs[:], dbp[:])
                        nc.vector.tensor_tensor(
                            yt_sb[hp][:, t0 + c * 512 : t0 + (c + 1) * 512],
                            av[c][:],
                            dbs[:],
                            op=Alu.mult,
                        )

                return finalize

            # ---------------- schedule (hp-major) ----------------
            # A=(hp0,th0) B=(hp1,th0) C=(hp0,th1) D=(hp1,th1).
            # Deadlines: k0cp1 by A-slot 7 (scores ub>=8); v(0,1) by A-slot
            # 8; q1cp0/k1cp0/v(1,0) by end of A (phase B); k1cp1 by B-slot
            # 7; v(1,1) by B-slot 8 (emitted in A); q0cp1 by end of B
            # (phase C); q1cp1 by end of C; fin_X as first filler of the
            # following phase.
            for _ in qk_chunk(0, "q", 0):
                pass
            for _ in qk_chunk(0, "k", 0):
                pass
            for _ in v_chunk(0, 0):
                pass
            # low-priority gap fillers on the (still idle) st banks: these
            # have no pj-ring dependencies, so the scheduler runs them
            # whenever the DMA-gated preamble leaves the PE idle, keeping
            # the HAM clock gate open until attention starts
            for i in range(12):
                wst = st_pool.tile(
                    [128, 1024], dt.float32, tag="st", name=f"warmst_{i}"
                )
                nc.tensor.matmul(
                    wst[:, 0:512], sel[0][:], warm_src[:], start=True, stop=True
                )

            qkc = lambda hp, w, c: (lambda: qk_chunk(hp, w, c))
            vc = lambda h, c: (lambda: v_chunk(h, c))
            fillers_a = [
                vc(0, 1),
                qkc(0, "k", 1),
                qkc(1, "q", 0),
                qkc(1, "k", 0),
                vc(1, 0),
                vc(1, 1),
            ]
            fin_a = attention_phase(0, 0, fillers_a)

            fillers_b = [fin_a, qkc(1, "k", 1), qkc(0, "q", 1)]
            fin_b = attention_phase(1, 0, fillers_b)

            fillers_c = [fin_b, qkc(1, "q", 1)] + [
                (lambda t=tb: out_chunk(t)) for tb in range(0, 8)
            ]
            fin_c = attention_phase(0, 1, fillers_c)

            fillers_d = [fin_c]
            fin_d = attention_phase(1, 1, fillers_d)
            # pj-ring position matters: these warm matmuls' slots are freed
            # by phase D's own output chunks, so they execute during the
            # fin_d normalize chain and keep the clock gate open for the tail
            warm_mms(6, "tail")
            fin_d()

            for tb in range(8, 16):
                out_chunk(tb, use_act=True)

    nc.finalize()
    return nc


def _prepare_in_maps(x, Wq, Wk, Wv, Wo):
    scale = 1.0 / math.sqrt(K)
    xT = [np.ascontiguousarray(x[b].T).astype(_BF16) for b in range(B)]
    in_maps = []
    for c in range(NCORES):
        b = c // 4
        g = c % 4
        sl = slice(g * 256, (g + 1) * 256)
        in_maps.append(
            {
                "xT": xT[b],
                "wq": np.ascontiguousarray((Wq[sl, :].astype(np.float64) * scale).T).astype(_BF16),
                "wk": np.ascontiguousarray(Wk[sl, :].T).astype(_BF16),
                "wv": np.ascontiguousarray(Wv[sl, :].T).astype(_BF16),
                "wo": np.ascontiguousarray(Wo[:, sl].T).astype(_BF16),
            }
        )
    return in_maps


def _gather(results, bo):
    out = np.zeros((B, T, K), dtype=np.float32)
    for b in range(B):
        acc = np.zeros((T, K), dtype=np.float32)
        for g in range(4):
            acc += results[b * 4 + g]["out"].astype(np.float32)
        out[b] = acc + bo.astype(np.float32)[None, :]
    return out


def _maybe_enable_ldw_opt():
    import os
    import concourse.bass_utils as bu

    if os.environ.get("LDWOPT", "0") != "1":
        return
    if getattr(bu, "_ldwopt_patched", False):
        return
    orig = bu.run_command

    def patched(argv, **kw):
        argv = [
            "--enable-ldw-opt=true" if a == "--enable-ldw-opt=false" else a
            for a in argv
        ]
        return orig(argv, **kw)

    bu.run_command = patched
    bu._ldwopt_patched = True


def run(x, Wq, Wk, Wv, Wo, bo, trace=False, tmpdir=None):
    from concourse.bass_utils import run_bass_kernel_spmd

    _maybe_enable_ldw_opt()
    _install_drain_split_patch()

    nc = build_program()
    in_maps = _prepare_in_maps(
        np.asarray(x), np.asarray(Wq), np.asarray(Wk), np.asarray(Wv), np.asarray(Wo)
    )
    res = run_bass_kernel_spmd(
        nc, in_maps, list(range(NCORES)), trace=trace, tmpdir=tmpdir
    )
    out = _gather(res.results, np.asarray(bo))
    return out, res


def kernel(x, Wq, Wk, Wv, Wo, bo):
    out, _ = run(x, Wq, Wk, Wv, Wo, bo, trace=False)
    return out
